# revision 14
# baseline (speedup 1.0000x reference)
"""Causal self-attention with RoPE on 8 Trainium2 NeuronCores.

Problem (hardcoded): B=2, S=2048, E=2048, H=16 heads, D=128 head dim.
  qkv = x @ W_qkv.T ; RoPE(q, k) ; causal softmax attention ; out @ W_out.T

Sharding: tensor-parallel over heads. Each of the 8 cores handles 2 heads
for both batches: computes its heads' q/k/v projections (column-sharded
W_qkv), runs attention, and produces a partial output projection
(row-sharded W_out). The host sums the 8 partial outputs.

Device-side structure (single software-pipelined loop, not phases):
 - Per 512-token block n: qkv projection for block n, then attention for
   unit n (the attention unit whose q-block is n; its k/v deps are blocks
   <= n, just produced), then the output projection of unit n-1. The
   dense qkv matmul stream gives the Tile scheduler independent PE work
   to hide every exp/normalize latency inside the attention pipeline.
 - q/k are produced feature-major ([D, token], the layout scores need);
   v is produced directly in [token, D] (PV lhsT layout) by swapping the
   matmul operands (lhsT = x chunk, rhs = Wv columns), which removes the
   PE-transpose pass entirely and frees its PSUM banks.
 - Attention computes transposed score tiles [k, q]; exp on ScalarE;
   causality by computing only k-tiles with k <= q (diagonal-band tiles
   get a {0,1} mask after exp, and are ragged: only q-columns >= the
   diagonal are computed).
 - Softmax denominators: the exp tiles of a (unit, head) are accumulated
   into one [128, 512] tile by a chain of VectorE adds (off the PE),
   then a single ones-column matmul reduces partitions. The reciprocal
   is broadcast across partitions via a DRAM bounce mid-loop (fully
   hidden), and via a rank-1 matmul broadcast for the last unit (the
   bounce's ~4us latency would be exposed there).
 - Output-projection partials are written to DRAM in bf16 (halves the
   output DMA and the PSUM-evacuation cost; the host sums in f32).
"""

import math
from contextlib import ExitStack

import numpy as np
import ml_dtypes

import concourse.bass as bass
import concourse.mybir as mybir
import concourse.tile as tile
from concourse import bacc
from concourse.bass_utils import run_bass_kernel_spmd

BF16 = mybir.dt.bfloat16
F32 = mybir.dt.float32
P = 128

# problem config
B, S, E = 2, 2048, 2048
H, D = 16, 128
N_CORES = 8
HPC = H // N_CORES  # heads per core = 2


def build_nc(b=B, s=S, e=E, hpc=HPC):
    """Build the per-core Bass program (same program on every core)."""
    T = b * s            # total tokens
    NT = T // 512        # 512-token blocks
    KE = e // P          # contraction tiles for the qkv projection
    QT = s // 512        # 512-wide q blocks per batch
    KT = s // P          # 128-wide k blocks per batch
    ME = e // P          # output-embedding tiles

    nc = bacc.Bacc("TRN2", target_bir_lowering=False, debug=False)

    xT = nc.dram_tensor("xT", [P, KE, T], BF16, kind="ExternalInput").ap()
    wqkv = nc.dram_tensor("wqkv", [P, KE, 3 * hpc * P], BF16, kind="ExternalInput").ap()
    wo = nc.dram_tensor("wo", [P, hpc, e], BF16, kind="ExternalInput").ap()
    cosq = nc.dram_tensor("cosq", [P, s], BF16, kind="ExternalInput").ap()
    sinq = nc.dram_tensor("sinq", [P, s], BF16, kind="ExternalInput").ap()
    cosk = nc.dram_tensor("cosk", [P, s], BF16, kind="ExternalInput").ap()
    sink = nc.dram_tensor("sink", [P, s], BF16, kind="ExternalInput").ap()
    bandmask = nc.dram_tensor("bandmask", [P, 896], BF16, kind="ExternalInput").ap()
    outT = nc.dram_tensor("outT", [e, T], BF16, kind="ExternalOutput").ap()

    with tile.TileContext(nc) as tc, ExitStack() as ctx:
        persist = ctx.enter_context(tc.tile_pool(name="persist", bufs=1))
        store = ctx.enter_context(tc.tile_pool(name="store", bufs=1))
        exp_pool = ctx.enter_context(tc.tile_pool(name="expp", bufs=8))
        small = ctx.enter_context(tc.tile_pool(name="small", bufs=3))
        opool = ctx.enter_context(tc.tile_pool(name="outp", bufs=5))
        dram_pool = ctx.enter_context(tc.tile_pool(name="drbounce", bufs=3, space="DRAM"))
        wpool = ctx.enter_context(tc.tile_pool(name="wq", bufs=1))
        xpool = ctx.enter_context(tc.tile_pool(name="xs", bufs=3))
        trig_pool = ctx.enter_context(tc.tile_pool(name="trig", bufs=1))
        rope_pool = ctx.enter_context(tc.tile_pool(name="rope", bufs=3))
        # PSUM: exactly 8 banks
        qkv_ps = ctx.enter_context(tc.tile_pool(name="qkvps", bufs=2, space="PSUM"))
        sc_ps = ctx.enter_context(tc.tile_pool(name="scps", bufs=1, space="PSUM"))
        att_ps = ctx.enter_context(tc.tile_pool(name="attps", bufs=2, space="PSUM"))
        out_ps = ctx.enter_context(tc.tile_pool(name="outps", bufs=2, space="PSUM"))

        ones_col = persist.tile([P, 1], BF16)
        nc.vector.memset(ones_col, 1.0)
        ones_row = persist.tile([1, P], BF16)
        nc.vector.memset(ones_row, 1.0)
        scratch = persist.tile([P, 512], BF16)
        nc.vector.memset(scratch, 0.0)
        # HAM warmup: keep the PE busy through the cold-clock window while
        # the first weight/x DMAs land, so real matmuls start at 2.4 GHz
        warm_ps = qkv_ps.tile([P, 512], F32, name="warmps", tag="ps")
        for i in range(10):
            nc.tensor.matmul(warm_ps, scratch[:, 0:P], scratch,
                             start=(i == 0), stop=(i == 9))
        mask_sb = persist.tile([P, 896], BF16)
        wo_sb = persist.tile([P, hpc, e], BF16)

        attn_sb = [store.tile([P, T], BF16, name=f"attnsb{h}") for h in range(hpc)]
        qk_sb = [store.tile([P, T], BF16, name=f"qksb{i}") for i in range(2 * hpc)]
        vblk = [store.tile([P, T // P, P], BF16, name=f"vblk{h}") for h in range(hpc)]

        # ---- initial DMAs (k-chunked so the first matmuls start early) ----
        w_sb = wpool.tile([P, KE, 3 * hpc * P], BF16)
        x_tiles = [None] * NT
        x_tiles[0] = xpool.tile([P, KE, 512], BF16, name="x_sb")
        quarter = 3 * hpc * P // 4
        for q4 in range(4):
            nc.sync.dma_start(w_sb[:, 0, q4 * quarter:(q4 + 1) * quarter],
                              wqkv[:, 0, q4 * quarter:(q4 + 1) * quarter])
            nc.sync.dma_start(x_tiles[0][:, 0, q4 * 128:(q4 + 1) * 128],
                              xT[:, 0, q4 * 128:(q4 + 1) * 128])
        for k in range(1, KE):
            nc.sync.dma_start(w_sb[:, k, :], wqkv[:, k, :])
            nc.sync.dma_start(x_tiles[0][:, k, :], xT[:, k, 0:512])
        trig = {}
        for nm, ap in [("cosq", cosq), ("sinq", sinq), ("cosk", cosk), ("sink", sink)]:
            t = trig_pool.tile([P, s], BF16, name=nm + "_sb")
            nc.sync.dma_start(t, ap)
            trig[nm] = t
        nc.sync.dma_start(mask_sb, bandmask)
        nc.sync.dma_start(wo_sb, wo)

        def emit_outproj(nt):
            for mt in range(ME):
                ps = out_ps.tile([P, 512], F32, name="ops")
                for h in range(hpc):
                    nc.tensor.matmul(
                        ps, wo_sb[:, h, mt * P:(mt + 1) * P],
                        attn_sb[h][:, nt * 512:(nt + 1) * 512],
                        start=(h == 0), stop=(h == hpc - 1),
                    )
                osb = opool.tile([P, 512], BF16, name="osb")
                if mt % 4 == 3:
                    nc.scalar.copy(out=osb, in_=ps)
                else:
                    nc.vector.tensor_copy(out=osb, in_=ps)
                nc.sync.dma_start(
                    outT[mt * P:(mt + 1) * P, nt * 512:(nt + 1) * 512], osb)

        prev_nt = None
        for n in range(NT):
            bb, qt = divmod(n, QT)
            s0 = qt * 512  # position offset within the batch

            # ---- qkv projection for block n ----
            x_sb = x_tiles[n]
            if x_sb is None:
                x_sb = xpool.tile([P, KE, 512], BF16, name="x_sb")
                for k in range(KE):
                    nc.sync.dma_start(x_sb[:, k, :],
                                      xT[:, k, n * 512:(n + 1) * 512])
            # q/k tiles, feature-major [D, tok], with RoPE
            for m in range(2 * hpc):
                ps = qkv_ps.tile([P, 512], F32, name="qkvps", tag="ps")
                for k in range(KE):
                    nc.tensor.matmul(
                        ps, w_sb[:, k, m * P:(m + 1) * P], x_sb[:, k, :],
                        start=(k == 0), stop=(k == KE - 1),
                    )
                kind, h = m // hpc, m % hpc
                raw = rope_pool.tile([P, 512], BF16, name="raw")
                nc.scalar.copy(out=raw, in_=ps)
                shuf = rope_pool.tile([P, 512], BF16, name="shuf")
                nc.gpsimd.tensor_copy(out=shuf[0:64], in_=raw[64:128])
                nc.gpsimd.tensor_copy(out=shuf[64:128], in_=raw[0:64])
                c_t = trig["cosq" if kind == 0 else "cosk"][:, s0:s0 + 512]
                s_t = trig["sinq" if kind == 0 else "sink"][:, s0:s0 + 512]
                t1 = rope_pool.tile([P, 512], BF16, name="t1")
                nc.vector.tensor_mul(t1, raw, c_t)
                nc.vector.tensor_mul(shuf, shuf, s_t)
                dst = qk_sb[kind * hpc + h][:, n * 512:(n + 1) * 512]
                nc.vector.tensor_add(dst, t1, shuf)
            # v directly in [token, D] layout: lhsT = x chunk, rhs = Wv cols
            for tb in range(4):
                vps = qkv_ps.tile([P, hpc * P], F32, name="vps", tag="ps")
                for k in range(KE):
                    nc.tensor.matmul(
                        vps, x_sb[:, k, tb * P:(tb + 1) * P],
                        w_sb[:, k, 2 * hpc * P:3 * hpc * P],
                        start=(k == 0), stop=(k == KE - 1),
                    )
                nc.scalar.copy(out=vblk[0][:, n * 4 + tb, :], in_=vps[:, 0:P])
                nc.vector.tensor_copy(out=vblk[1][:, n * 4 + tb, :],
                                      in_=vps[:, P:2 * P])

            # ---- attention unit n: q-block qt of batch bb ----
            nk = 4 * (qt + 1)  # causal: k-tiles 0 .. 4(qt+1)-1
            last_unit = (n == NT - 1)
            att_t, rb_t = [], []
            for h in range(hpc):
                q_store = qk_sb[h]
                k_store = qk_sb[hpc + h]
                att = att_ps.tile([P, 512], F32, name="att")
                # denominators: 4 col-group-packed ones-matmul accumulators
                # (rows 0/32/64/96; group g sums k-tiles with kt%4 == g).
                # The 4 packed matmuls of a quad run concurrently on the PE.
                sm_t = sc_ps.tile([P, 512], F32, name="smt", tag="sm")
                qsl = q_store[:, bb * s + qt * 512: bb * s + (qt + 1) * 512]
                e_tiles = []
                for kt in range(nk):
                    # causal raggedness: diagonal-band tile j only needs
                    # q-columns >= 128*j
                    j = kt - 4 * qt
                    off = max(0, 128 * j)
                    w_q = 512 - off
                    sp = sc_ps.tile([P, 512], F32, name="sp", tag="sc")
                    nc.tensor.matmul(
                        sp[:, :w_q],
                        k_store[:, bb * s + kt * P: bb * s + (kt + 1) * P],
                        qsl[:, off:512], start=True, stop=True,
                    )
                    e_t = exp_pool.tile([P, 512], BF16, name="e_t")
                    nc.scalar.activation(
                        e_t[:, :w_q], sp[:, :w_q],
                        mybir.ActivationFunctionType.Exp)
                    if j >= 0:  # triangle block only
                        nc.vector.tensor_mul(
                            e_t[:, 0:128], e_t[:, 0:128], mask_sb[:, 384:512])
                    nc.tensor.matmul(
                        att[:, off:512], vblk[h][:, bb * KT + kt, :],
                        e_t[:, :w_q],
                        start=(kt == 0), stop=(kt == nk - 1),
                    )
                    e_tiles.append((e_t, off, w_q))
                    if kt % 4 == 3:  # emit a packed quad of ones-matmuls
                        for g in range(4):
                            kt_g = kt - 3 + g
                            et, off_g, wq_g = e_tiles[kt_g]
                            nc.tensor.matmul(
                                sm_t[32 * g:32 * g + 1, off_g:512],
                                ones_col, et[:, :wq_g],
                                start=(kt_g < 4), stop=(kt_g >= nk - 4),
                                tile_position=(0, 32 * g),
                            )
                # combine the 4 group rows (group g is valid on [vg:512])
                den = small.tile([1, 512], F32, name="den")
                nc.vector.tensor_copy(out=den, in_=sm_t[0:1, :])
                for g in range(1, 4):
                    vg = 128 * g if qt == 0 else 0
                    nc.vector.tensor_tensor(
                        den[:, vg:512], den[:, vg:512],
                        sm_t[32 * g:32 * g + 1, vg:512], mybir.AluOpType.add)
                r = small.tile([1, 512], F32, name="r")
                nc.vector.reciprocal_approx_fast(out=r, in_=den)
                rb = small.tile([P, 512], F32, name="rb")
                if not last_unit:
                    # broadcast across partitions via a DRAM bounce (DMA can
                    # replicate a DRAM row with a zero-stride partition dim);
                    # ~4us latency, fully hidden behind the pipelined outproj
                    rd = dram_pool.tile([1, 512], F32, name="rd")
                    nc.sync.dma_start(rd, r)
                    rd_b = bass.AP(tensor=rd.tensor, offset=rd.offset,
                                   ap=[[0, P]] + list(rd.ap[1:]))
                    nc.sync.dma_start(rb, rd_b)
                else:
                    # tail: rank-1 matmul broadcast (no 4us bounce latency)
                    r8 = small.tile([1, 512], BF16, name="r8")
                    nc.vector.tensor_copy(out=r8, in_=r)
                    rbp = qkv_ps.tile([P, 512], F32, name="rbp", tag="ps")
                    nc.tensor.matmul(rbp, ones_row, r8, start=True, stop=True)
                    nc.vector.tensor_copy(out=rb, in_=rbp)
                att_t.append(att)
                rb_t.append(rb)
            # outproj of the previous unit goes FIRST so its PSUM->SBUF
            # copies aren't queued behind the norms (which wait on the
            # broadcast) in the in-order DVE FIFO
            if prev_nt is not None:
                emit_outproj(prev_nt)
            for h in range(hpc):
                nc.vector.tensor_tensor(
                    attn_sb[h][:, bb * s + qt * 512: bb * s + (qt + 1) * 512],
                    att_t[h], rb_t[h], mybir.AluOpType.mult,
                )
            prev_nt = n
        emit_outproj(prev_nt)

    nc.compile()
    return nc


def make_common_inputs(x, b=B, s=S, e=E):
    """Inputs identical on every core: xT, trig tables, causal band mask."""
    T = b * s
    KE = e // P
    xflat = np.ascontiguousarray(x.reshape(T, e).T)        # [E, T] f32
    xT = np.ascontiguousarray(
        xflat.reshape(KE, P, T).transpose(1, 0, 2)).astype(ml_dtypes.bfloat16)

    inv_freq = (1.0 / (10000.0 ** (np.arange(0, D, 2, dtype=np.float32) / D)))
    t = np.arange(s, dtype=np.float32)
    freqs = np.outer(t, inv_freq)                           # [S, 64]
    cos = np.cos(freqs).astype(np.float32)                  # [S, 64]
    sin = np.sin(freqs).astype(np.float32)
    cosT = np.concatenate([cos, cos], axis=1).T             # [128, S]
    sinT = np.concatenate([sin, sin], axis=1).T
    sgn = np.where(np.arange(D) < D // 2, -1.0, 1.0).astype(np.float32)[:, None]
    scale = 1.0 / math.sqrt(D)
    cosq = np.ascontiguousarray(cosT * scale).astype(ml_dtypes.bfloat16)
    sinq = np.ascontiguousarray(sinT * sgn * scale).astype(ml_dtypes.bfloat16)
    cosk = np.ascontiguousarray(cosT).astype(ml_dtypes.bfloat16)
    sink = np.ascontiguousarray(sinT * sgn).astype(ml_dtypes.bfloat16)

    r = np.arange(P)[:, None]
    cc = np.arange(896)[None, :]
    bandmask = (cc >= r + 384).astype(ml_dtypes.bfloat16)

    return {
        "xT": xT, "cosq": cosq, "sinq": sinq, "cosk": cosk, "sink": sink,
        "bandmask": bandmask,
    }


def make_core_inputs(W_qkv, W_out, core, b=B, s=S, e=E, hpc=HPC):
    """Per-core column-sharded W_qkv (as lhsT tiles) and row-sharded W_out."""
    KE = e // P
    heads = [core * hpc + i for i in range(hpc)]
    rows = []
    for base in (0, e, 2 * e):  # q, k, v row blocks of W_qkv
        for h in heads:
            rows.append(W_qkv[base + h * D: base + (h + 1) * D])
    Wc = np.concatenate(rows, axis=0)                       # [3*hpc*128, E]
    WcT = np.ascontiguousarray(Wc.T)                        # [E, 3*hpc*128]
    wqkv = np.ascontiguousarray(
        WcT.reshape(KE, P, 3 * hpc * P).transpose(1, 0, 2)).astype(ml_dtypes.bfloat16)

    wo = np.stack(
        [np.ascontiguousarray(W_out[:, h * D:(h + 1) * D].T) for h in heads],
        axis=1)                                             # [128, hpc, E]
    wo = np.ascontiguousarray(wo).astype(ml_dtypes.bfloat16)
    return {"wqkv": wqkv, "wo": wo}


_NC_CACHE = {}


def get_nc():
    key = (B, S, E, HPC)
    if key not in _NC_CACHE:
        _NC_CACHE[key] = build_nc()
    return _NC_CACHE[key]


def kernel(x, W_qkv, W_out):
    x = np.asarray(x, dtype=np.float32)
    W_qkv = np.asarray(W_qkv, dtype=np.float32)
    W_out = np.asarray(W_out, dtype=np.float32)

    nc = get_nc()
    common = make_common_inputs(x)
    in_maps = [dict(common, **make_core_inputs(W_qkv, W_out, c))
               for c in range(N_CORES)]
    res = run_bass_kernel_spmd(nc, in_maps, list(range(N_CORES)))
    total = res.results[0]["outT"].astype(np.float32)
    for c in range(1, N_CORES):
        total = total + res.results[c]["outT"].astype(np.float32)
    return np.ascontiguousarray(total.T).reshape(B, S, E).astype(np.float32)


# revision 19
# speedup vs baseline: 1.2686x; 1.2686x over previous
"""Causal self-attention with RoPE on 8 Trainium2 NeuronCores.

Problem (hardcoded): B=2, S=2048, E=2048, H=16 heads, D=128 head dim.
  qkv = x @ W_qkv.T ; RoPE(q, k) ; causal softmax attention ; out @ W_out.T

Sharding: tensor-parallel over heads. Each of the 8 cores handles 2 heads
for both batches: computes its heads' q/k/v projections (column-sharded
W_qkv), runs attention, and produces a partial output projection
(row-sharded W_out). The host sums the 8 partial outputs.

Device-side structure (single software-pipelined loop, not phases):
 - Per 512-token block n: qkv projection for block n, then attention for
   unit n (the attention unit whose q-block is n; its k/v deps are blocks
   <= n, just produced), then the output projection of unit n-1. The
   dense qkv matmul stream gives the Tile scheduler independent PE work
   to hide every exp/normalize latency inside the attention pipeline.
 - q/k are produced feature-major ([D, token], the layout scores need);
   v is produced directly in [token, D] (PV lhsT layout) by swapping the
   matmul operands (lhsT = x chunk, rhs = Wv columns), which removes the
   PE-transpose pass entirely and frees its PSUM banks.
 - Attention computes transposed score tiles [k, q]; exp on ScalarE;
   causality by computing only k-tiles with k <= q (diagonal-band tiles
   get a {0,1} mask after exp, and are ragged: only q-columns >= the
   diagonal are computed).
 - Softmax denominators: the exp tiles of a (unit, head) are accumulated
   into one [128, 512] tile by a chain of VectorE adds (off the PE),
   then a single ones-column matmul reduces partitions. The reciprocal
   is broadcast across partitions via a DRAM bounce mid-loop (fully
   hidden), and via a rank-1 matmul broadcast for the last unit (the
   bounce's ~4us latency would be exposed there).
 - Output-projection partials are written to DRAM in bf16 (halves the
   output DMA and the PSUM-evacuation cost; the host sums in f32).
"""

import math
from contextlib import ExitStack

import numpy as np
import ml_dtypes

import concourse.bass as bass
import concourse.mybir as mybir
import concourse.tile as tile
from concourse import bacc
from concourse.bass_utils import run_bass_kernel_spmd

BF16 = mybir.dt.bfloat16
F32 = mybir.dt.float32
P = 128

# problem config
B, S, E = 2, 2048, 2048
H, D = 16, 128
N_CORES = 8
HPC = H // N_CORES  # heads per core = 2


def build_nc(b=B, s=S, e=E, hpc=HPC):
    """Build the per-core Bass program (same program on every core)."""
    T = b * s            # total tokens
    NT = T // 512        # 512-token blocks
    KE = e // P          # contraction tiles for the qkv projection
    QT = s // 512        # 512-wide q blocks per batch
    KT = s // P          # 128-wide k blocks per batch
    ME = e // P          # output-embedding tiles

    nc = bacc.Bacc("TRN2", target_bir_lowering=False, debug=False)

    xT = nc.dram_tensor("xT", [P, KE, T], BF16, kind="ExternalInput").ap()
    wqkv = nc.dram_tensor("wqkv", [P, KE, 3 * hpc * P], BF16, kind="ExternalInput").ap()
    wo = nc.dram_tensor("wo", [P, hpc, e], BF16, kind="ExternalInput").ap()
    cosq = nc.dram_tensor("cosq", [P, s], BF16, kind="ExternalInput").ap()
    sinq = nc.dram_tensor("sinq", [P, s], BF16, kind="ExternalInput").ap()
    cosk = nc.dram_tensor("cosk", [P, s], BF16, kind="ExternalInput").ap()
    sink = nc.dram_tensor("sink", [P, s], BF16, kind="ExternalInput").ap()
    bandmask = nc.dram_tensor("bandmask", [P, 896], BF16, kind="ExternalInput").ap()
    outT = nc.dram_tensor("outT", [e, T], BF16, kind="ExternalOutput").ap()

    with tile.TileContext(nc) as tc, ExitStack() as ctx:
        persist = ctx.enter_context(tc.tile_pool(name="persist", bufs=1))
        store = ctx.enter_context(tc.tile_pool(name="store", bufs=1))
        exp_pool = ctx.enter_context(tc.tile_pool(name="expp", bufs=8))
        acc_pool = ctx.enter_context(tc.tile_pool(name="accp", bufs=3))
        small = ctx.enter_context(tc.tile_pool(name="small", bufs=3))
        opool = ctx.enter_context(tc.tile_pool(name="outp", bufs=5))
        dram_pool = ctx.enter_context(tc.tile_pool(name="drbounce", bufs=3, space="DRAM"))
        wpool = ctx.enter_context(tc.tile_pool(name="wq", bufs=1))
        xpool = ctx.enter_context(tc.tile_pool(name="xs", bufs=3))
        trig_pool = ctx.enter_context(tc.tile_pool(name="trig", bufs=1))
        rope_pool = ctx.enter_context(tc.tile_pool(name="rope", bufs=3))
        # PSUM: exactly 8 banks
        qkv_ps = ctx.enter_context(tc.tile_pool(name="qkvps", bufs=2, space="PSUM"))
        sc_ps = ctx.enter_context(tc.tile_pool(name="scps", bufs=2, space="PSUM"))
        att_ps = ctx.enter_context(tc.tile_pool(name="attps", bufs=2, space="PSUM"))
        out_ps = ctx.enter_context(tc.tile_pool(name="outps", bufs=2, space="PSUM"))

        ones_col = persist.tile([P, 1], BF16)
        nc.vector.memset(ones_col, 1.0)
        ones_row = persist.tile([1, P], BF16)
        nc.vector.memset(ones_row, 1.0)
        scratch = persist.tile([P, 512], BF16)
        nc.vector.memset(scratch, 0.0)
        # HAM warmup: keep the PE busy through the cold-clock window while
        # the first weight/x DMAs land, so real matmuls start at 2.4 GHz
        warm_ps = qkv_ps.tile([P, 512], F32, name="warmps", tag="ps")
        for i in range(10):
            nc.tensor.matmul(warm_ps, scratch[:, 0:P], scratch,
                             start=(i == 0), stop=(i == 9))
        mask_sb = persist.tile([P, 896], BF16)
        wo_sb = persist.tile([P, hpc, e], BF16)

        attn_sb = [store.tile([P, T], BF16, name=f"attnsb{h}") for h in range(hpc)]
        qk_sb = [store.tile([P, T], BF16, name=f"qksb{i}") for i in range(2 * hpc)]
        vblk = [store.tile([P, T // P, P], BF16, name=f"vblk{h}") for h in range(hpc)]

        # ---- initial DMAs (k-chunked so the first matmuls start early) ----
        w_sb = wpool.tile([P, KE, 3 * hpc * P], BF16)
        x_tiles = [None] * NT
        x_tiles[0] = xpool.tile([P, KE, 512], BF16, name="x_sb")
        quarter = 3 * hpc * P // 4
        for q4 in range(4):
            nc.sync.dma_start(w_sb[:, 0, q4 * quarter:(q4 + 1) * quarter],
                              wqkv[:, 0, q4 * quarter:(q4 + 1) * quarter])
            nc.sync.dma_start(x_tiles[0][:, 0, q4 * 128:(q4 + 1) * 128],
                              xT[:, 0, q4 * 128:(q4 + 1) * 128])
        for k in range(1, KE):
            nc.sync.dma_start(w_sb[:, k, :], wqkv[:, k, :])
            nc.sync.dma_start(x_tiles[0][:, k, :], xT[:, k, 0:512])
        trig = {}
        for nm, ap in [("cosq", cosq), ("sinq", sinq), ("cosk", cosk), ("sink", sink)]:
            t = trig_pool.tile([P, s], BF16, name=nm + "_sb")
            nc.sync.dma_start(t, ap)
            trig[nm] = t
        nc.sync.dma_start(mask_sb, bandmask)
        nc.sync.dma_start(wo_sb, wo)

        def emit_outproj(nt):
            for mt in range(ME):
                ps = out_ps.tile([P, 512], F32, name="ops")
                for h in range(hpc):
                    nc.tensor.matmul(
                        ps, wo_sb[:, h, mt * P:(mt + 1) * P],
                        attn_sb[h][:, nt * 512:(nt + 1) * 512],
                        start=(h == 0), stop=(h == hpc - 1),
                    )
                osb = opool.tile([P, 512], BF16, name="osb")
                if mt % 4 == 3:
                    nc.scalar.copy(out=osb, in_=ps)
                else:
                    nc.vector.tensor_copy(out=osb, in_=ps)
                nc.sync.dma_start(
                    outT[mt * P:(mt + 1) * P, nt * 512:(nt + 1) * 512], osb)

        prev_nt = None
        for n in range(NT):
            bb, qt = divmod(n, QT)
            s0 = qt * 512  # position offset within the batch

            # ---- qkv projection for block n ----
            x_sb = x_tiles[n]
            if x_sb is None:
                x_sb = xpool.tile([P, KE, 512], BF16, name="x_sb")
                for k in range(KE):
                    nc.sync.dma_start(x_sb[:, k, :],
                                      xT[:, k, n * 512:(n + 1) * 512])
            # q/k tiles, feature-major [D, tok], with RoPE
            for m in range(2 * hpc):
                ps = qkv_ps.tile([P, 512], F32, name="qkvps", tag="ps")
                for k in range(KE):
                    nc.tensor.matmul(
                        ps, w_sb[:, k, m * P:(m + 1) * P], x_sb[:, k, :],
                        start=(k == 0), stop=(k == KE - 1),
                    )
                kind, h = m // hpc, m % hpc
                raw = rope_pool.tile([P, 512], BF16, name="raw")
                nc.scalar.copy(out=raw, in_=ps)
                shuf = rope_pool.tile([P, 512], BF16, name="shuf")
                nc.vector.tensor_copy(out=shuf[0:64], in_=raw[64:128])
                nc.vector.tensor_copy(out=shuf[64:128], in_=raw[0:64])
                c_t = trig["cosq" if kind == 0 else "cosk"][:, s0:s0 + 512]
                s_t = trig["sinq" if kind == 0 else "sink"][:, s0:s0 + 512]
                t1 = rope_pool.tile([P, 512], BF16, name="t1")
                nc.vector.tensor_mul(t1, raw, c_t)
                nc.vector.tensor_mul(shuf, shuf, s_t)
                dst = qk_sb[kind * hpc + h][:, n * 512:(n + 1) * 512]
                nc.vector.tensor_add(dst, t1, shuf)
            # v directly in [token, D] layout: lhsT = x chunk, rhs = Wv cols
            for tb in range(4):
                vps = qkv_ps.tile([P, hpc * P], F32, name="vps", tag="ps")
                for k in range(KE):
                    nc.tensor.matmul(
                        vps, x_sb[:, k, tb * P:(tb + 1) * P],
                        w_sb[:, k, 2 * hpc * P:3 * hpc * P],
                        start=(k == 0), stop=(k == KE - 1),
                    )
                nc.scalar.copy(out=vblk[0][:, n * 4 + tb, :], in_=vps[:, 0:P])
                nc.vector.tensor_copy(out=vblk[1][:, n * 4 + tb, :],
                                      in_=vps[:, P:2 * P])

            # ---- attention unit n: q-block qt of batch bb ----
            nk = 4 * (qt + 1)  # causal: k-tiles 0 .. 4(qt+1)-1
            last_unit = (n == NT - 1)
            att_t, rb_t = [], []
            for h in range(hpc):
                q_store = qk_sb[h]
                k_store = qk_sb[hpc + h]
                att = att_ps.tile([P, 512], F32, name="att")
                acc = acc_pool.tile([P, 512], BF16, name="acc")
                qsl = q_store[:, bb * s + qt * 512: bb * s + (qt + 1) * 512]
                for kt in range(nk):
                    # causal raggedness: diagonal-band tile j only needs
                    # q-columns >= 128*j
                    j = kt - 4 * qt
                    off = max(0, 128 * j)
                    w_q = 512 - off
                    sp = sc_ps.tile([P, 512], F32, name="sp", tag="sc")
                    nc.tensor.matmul(
                        sp[:, :w_q],
                        k_store[:, bb * s + kt * P: bb * s + (kt + 1) * P],
                        qsl[:, off:512], start=True, stop=True,
                    )
                    e_t = exp_pool.tile([P, 512], BF16, name="e_t")
                    nc.scalar.activation(
                        e_t[:, :w_q], sp[:, :w_q],
                        mybir.ActivationFunctionType.Exp)
                    if j >= 0:  # triangle block only
                        nc.vector.tensor_mul(
                            e_t[:, 0:128], e_t[:, 0:128], mask_sb[:, 384:512])
                    nc.tensor.matmul(
                        att[:, off:512], vblk[h][:, bb * KT + kt, :],
                        e_t[:, :w_q],
                        start=(kt == 0), stop=(kt == nk - 1),
                    )
                    # denominator accumulation off the PE (VectorE chain)
                    if kt == 0:
                        nc.vector.tensor_copy(out=acc, in_=e_t)
                    else:
                        nc.vector.tensor_tensor(
                            acc[:, off:512], acc[:, off:512], e_t[:, :w_q],
                            mybir.AluOpType.add)
                # single partition-reduce matmul for the denominators
                sm = sc_ps.tile([1, 512], F32, name="sm", tag="sc")
                nc.tensor.matmul(sm, ones_col, acc, start=True, stop=True)
                r = small.tile([1, 512], F32, name="r")
                nc.vector.reciprocal_approx_fast(out=r, in_=sm)
                rb = small.tile([P, 512], F32, name="rb")
                if not last_unit:
                    # broadcast across partitions via a DRAM bounce (DMA can
                    # replicate a DRAM row with a zero-stride partition dim);
                    # ~4us latency, fully hidden behind the pipelined outproj
                    rd = dram_pool.tile([1, 512], F32, name="rd")
                    nc.sync.dma_start(rd, r)
                    rd_b = bass.AP(tensor=rd.tensor, offset=rd.offset,
                                   ap=[[0, P]] + list(rd.ap[1:]))
                    nc.sync.dma_start(rb, rd_b)
                else:
                    # tail: rank-1 matmul broadcast (no 4us bounce latency)
                    r8 = small.tile([1, 512], BF16, name="r8")
                    nc.vector.tensor_copy(out=r8, in_=r)
                    rbp = qkv_ps.tile([P, 512], F32, name="rbp", tag="ps")
                    nc.tensor.matmul(rbp, ones_row, r8, start=True, stop=True)
                    nc.vector.tensor_copy(out=rb, in_=rbp)
                att_t.append(att)
                rb_t.append(rb)
            # outproj of the previous unit goes FIRST so its PSUM->SBUF
            # copies aren't queued behind the norms (which wait on the
            # broadcast) in the in-order DVE FIFO
            if prev_nt is not None:
                emit_outproj(prev_nt)
            for h in range(hpc):
                nc.vector.tensor_tensor(
                    attn_sb[h][:, bb * s + qt * 512: bb * s + (qt + 1) * 512],
                    att_t[h], rb_t[h], mybir.AluOpType.mult,
                )
            prev_nt = n
        emit_outproj(prev_nt)

    nc.compile()
    return nc


def make_common_inputs(x, b=B, s=S, e=E):
    """Inputs identical on every core: xT, trig tables, causal band mask."""
    T = b * s
    KE = e // P
    xflat = np.ascontiguousarray(x.reshape(T, e).T)        # [E, T] f32
    xT = np.ascontiguousarray(
        xflat.reshape(KE, P, T).transpose(1, 0, 2)).astype(ml_dtypes.bfloat16)

    inv_freq = (1.0 / (10000.0 ** (np.arange(0, D, 2, dtype=np.float32) / D)))
    t = np.arange(s, dtype=np.float32)
    freqs = np.outer(t, inv_freq)                           # [S, 64]
    cos = np.cos(freqs).astype(np.float32)                  # [S, 64]
    sin = np.sin(freqs).astype(np.float32)
    cosT = np.concatenate([cos, cos], axis=1).T             # [128, S]
    sinT = np.concatenate([sin, sin], axis=1).T
    sgn = np.where(np.arange(D) < D // 2, -1.0, 1.0).astype(np.float32)[:, None]
    scale = 1.0 / math.sqrt(D)
    cosq = np.ascontiguousarray(cosT * scale).astype(ml_dtypes.bfloat16)
    sinq = np.ascontiguousarray(sinT * sgn * scale).astype(ml_dtypes.bfloat16)
    cosk = np.ascontiguousarray(cosT).astype(ml_dtypes.bfloat16)
    sink = np.ascontiguousarray(sinT * sgn).astype(ml_dtypes.bfloat16)

    r = np.arange(P)[:, None]
    cc = np.arange(896)[None, :]
    bandmask = (cc >= r + 384).astype(ml_dtypes.bfloat16)

    return {
        "xT": xT, "cosq": cosq, "sinq": sinq, "cosk": cosk, "sink": sink,
        "bandmask": bandmask,
    }


def make_core_inputs(W_qkv, W_out, core, b=B, s=S, e=E, hpc=HPC):
    """Per-core column-sharded W_qkv (as lhsT tiles) and row-sharded W_out."""
    KE = e // P
    heads = [core * hpc + i for i in range(hpc)]
    rows = []
    for base in (0, e, 2 * e):  # q, k, v row blocks of W_qkv
        for h in heads:
            rows.append(W_qkv[base + h * D: base + (h + 1) * D])
    Wc = np.concatenate(rows, axis=0)                       # [3*hpc*128, E]
    WcT = np.ascontiguousarray(Wc.T)                        # [E, 3*hpc*128]
    wqkv = np.ascontiguousarray(
        WcT.reshape(KE, P, 3 * hpc * P).transpose(1, 0, 2)).astype(ml_dtypes.bfloat16)

    wo = np.stack(
        [np.ascontiguousarray(W_out[:, h * D:(h + 1) * D].T) for h in heads],
        axis=1)                                             # [128, hpc, E]
    wo = np.ascontiguousarray(wo).astype(ml_dtypes.bfloat16)
    return {"wqkv": wqkv, "wo": wo}


_NC_CACHE = {}


def get_nc():
    key = (B, S, E, HPC)
    if key not in _NC_CACHE:
        _NC_CACHE[key] = build_nc()
    return _NC_CACHE[key]


def kernel(x, W_qkv, W_out):
    x = np.asarray(x, dtype=np.float32)
    W_qkv = np.asarray(W_qkv, dtype=np.float32)
    W_out = np.asarray(W_out, dtype=np.float32)

    nc = get_nc()
    common = make_common_inputs(x)
    in_maps = [dict(common, **make_core_inputs(W_qkv, W_out, c))
               for c in range(N_CORES)]
    res = run_bass_kernel_spmd(nc, in_maps, list(range(N_CORES)))
    total = res.results[0]["outT"].astype(np.float32)
    for c in range(1, N_CORES):
        total = total + res.results[c]["outT"].astype(np.float32)
    return np.ascontiguousarray(total.T).reshape(B, S, E).astype(np.float32)
